# revision 29
# baseline (speedup 1.0000x reference)
"""Trainium2 Bass kernel for a causal self-attention block (GQA + per-head
RMS-norm + RoPE + learned q-gain), sharded over 8 NeuronCores.

Sharding: data-parallel over batch (B=2) x tensor-parallel over KV groups
(4 groups of 4 query heads). core = b*4 + g. Each core computes full
attention for its 4 heads and a partial output projection (its 256 in-dims
of Wproj); the host sums the 4 partials per batch element.

v3 design:
- bf16 operands everywhere (fp32 PSUM accumulate).
- Scores as S^T[k, q] = K @ Q^T with heads PAIRED: two K=64 matmuls run
  concurrently in the PE via tile_position row groups.
- Causal trimming at 128-column granularity; one [128,128] step mask for
  the diagonal boundary.
- Phase-1 transposes: ONE fused DMA XBAR transpose per s-tile
  ([128,384] -> [128,3,128] block transpose) writing q-pair/k slabs
  directly into the merged qkT layout. Zero PE/DVE cost, 16 DMA ops.
- RMS rsqrt via Ln+Exp so the whole kernel uses a single ACT table set
  (natural_log_exp_and_others covers Exp/Ln/Square/Copy).
- Softmax denominators ride the PV matmul as a ones-column, gathered
  16-per-partition, one wide reciprocal, broadcast back via K=16 selector
  matmuls, normalization fused into the output-projection preamble.
- Phase interleaving keeps every engine queue dense.
"""

import math

import numpy as np
import ml_dtypes

import concourse.bacc as bacc
import concourse.tile as tile
from concourse import mybir
from concourse.bass import ts
from concourse.bass_utils import run_bass_kernel_spmd

# Problem dims (hardcoded per contract).
B, S, D, H, KV, HD = 2, 2048, 1024, 16, 4, 64
NH = H // KV          # 4 query heads per core (one KV group)
GD = NH * HD          # 256 out-dims of Wq per group
NQKV = GD + 2 * HD    # 384
P = 128               # partitions
NST = S // P          # 16 sequence tiles
JW = 512              # query-block width for attention
NJ = S // JW          # 4 query blocks
NC = 8                # cores
ROPE_BASE = 10000.0
RMS_EPS = 1.1920929e-07
F32 = mybir.dt.float32
BF16 = mybir.dt.bfloat16
AXX = mybir.AxisListType.X
ACT = mybir.ActivationFunctionType
ALU = mybir.AluOpType

bfloat16 = ml_dtypes.bfloat16


def _build_program():
    nc = bacc.Bacc("TRN2", target_bir_lowering=False, debug=False)

    xT = nc.dram_tensor("xT", [2 * D, S // 2], BF16, kind="ExternalInput").ap()
    wqkv = nc.dram_tensor("wqkv", [D, NQKV], BF16, kind="ExternalInput").ap()
    wp2 = nc.dram_tensor("wp2", [P, 2 * D], BF16, kind="ExternalInput").ap()
    cosn = nc.dram_tensor("cosn", [P, NST * HD], BF16, kind="ExternalInput").ap()
    sinpm = nc.dram_tensor("sinpm", [P, NST * HD], BF16, kind="ExternalInput").ap()
    maskt = nc.dram_tensor("maskt", [P, P], BF16, kind="ExternalInput").ap()
    selq = nc.dram_tensor("selq", [8, NJ * P], BF16, kind="ExternalInput").ap()
    qg8 = nc.dram_tensor("qg8", [1, NH], BF16, kind="ExternalInput").ap()
    ypt = nc.dram_tensor("ypt", [D, S], BF16, kind="ExternalOutput").ap()

    with tile.TileContext(nc) as tc:
        _body(tc, xT, wqkv, wp2, cosn, sinpm, maskt, selq, qg8, ypt)
    nc.compile()
    return nc


def _body(tc, xT, wqkv, wp2, cosn, sinpm, maskt, selq, qg8, ypt):
    nc = tc.nc

    with (
        tc.tile_pool(name="consts", bufs=1) as consts,
        tc.tile_pool(name="work", bufs=4) as work,
        tc.tile_pool(name="p2p", bufs=6) as p2p,
        tc.tile_pool(name="mmp", bufs=2, space="PSUM") as mmp,
        tc.tile_pool(name="stp", bufs=2, space="PSUM") as stp,
        tc.tile_pool(name="yp", bufs=1, space="PSUM") as yp,
    ):
        # ---------------- persistent SBUF state ----------------
        xT_sb = [
            [consts.tile([P, S // 2], BF16, name=f"xT_sb{c}_{hf}") for hf in range(2)]
            for c in range(8)
        ]
        wqkv_sb = consts.tile([P, 8, NQKV], BF16, name="wqkv_sb")
        wp_sb = consts.tile([P, 2, D], BF16, name="wp_sb")
        cos_sb = consts.tile([P, NST, HD], BF16, name="cos_sb")
        sin_sb = consts.tile([P, NST, HD], BF16, name="sin_sb")
        mask_sb = consts.tile([P, P], BF16, name="mask_sb")
        selq_sb = consts.tile([8, NJ, P], BF16, name="selq_sb")
        qg8_sb = consts.tile([P, NH], BF16, name="qg8_sb")
        # merged attention operand layout: [:, i, 0/1, :] = qT pair c tile i,
        # [:, i, 2, :] = kT duplicated into both partition halves.
        qkT_sb = consts.tile([P, NST, 3, P], BF16, name="qkT_sb")
        v_sb = consts.tile([P, NST, HD + 1], BF16, name="v_sb")
        y_sb = consts.tile([P, 2, S], BF16, name="y_sb")
        den_sb = consts.tile([64, NJ, P], BF16, name="den_sb")

        nc.sync.dma_start(out=wqkv_sb, in_=wqkv.rearrange("(c p) n -> p c n", p=P))
        for hf in range(2):
            for c in range(8):
                eng = nc.sync if c % 2 == 0 else nc.scalar
                base = hf * D + c * P
                eng.dma_start(out=xT_sb[c][hf], in_=xT[base : base + P, :])
        nc.scalar.dma_start(out=cos_sb, in_=cosn.rearrange("p (t f) -> p t f", f=HD))
        nc.scalar.dma_start(out=sin_sb, in_=sinpm.rearrange("p (t f) -> p t f", f=HD))
        nc.sync.dma_start(out=wp_sb, in_=wp2.rearrange("p (c m) -> p c m", c=2))
        nc.sync.dma_start(out=mask_sb, in_=maskt)
        nc.sync.dma_start(
            out=selq_sb, in_=selq.rearrange("r (j p) -> r j p", j=NJ)
        )
        nc.gpsimd.dma_start(out=qg8_sb, in_=qg8.to_broadcast([P, NH]))
        nc.vector.memset(v_sb[:, :, HD : HD + 1], 1.0)

        # ------------ phase 1: QKV + RMS + RoPE + transpose (tile pairs) --
        def phase1(u):
            i0 = 2 * u
            qkvs = []
            for k in range(2):
                qkv = mmp.tile([P, 512], F32, name=f"qkv{u}{k}", tag="mm")
                for cc in range(8):
                    nc.tensor.matmul(
                        qkv[:, 0:NQKV],
                        lhsT=xT_sb[cc][(i0 + k) // 8][:, ts((i0 + k) % 8, P)],
                        rhs=wqkv_sb[:, cc, :],
                        start=(cc == 0),
                        stop=(cc == 7),
                    )
                qkvs.append(qkv)
            qc = work.tile([P, 2, 5, HD], BF16, name=f"qc{u}", tag="qc")
            for k in range(2):
                nc.vector.tensor_copy(v_sb[:, i0 + k, 0:HD], qkvs[k][:, 5 * HD : NQKV])
                nc.vector.tensor_copy(
                    qc[:, k], qkvs[k][:, 0 : 5 * HD].rearrange("p (s d) -> p s d", d=HD)
                )
            sq = work.tile([P, 2, 5, HD], BF16, name=f"sq{u}", tag="sq")
            nc.vector.tensor_mul(sq, qc, qc)
            ss = work.tile([P, 2, 5], F32, name=f"ss{u}", tag="ss")
            nc.vector.reduce_sum(ss, sq, axis=AXX)
            m10 = work.tile([P, 2, 5], F32, name=f"m10{u}", tag="m10")
            nc.vector.tensor_scalar(
                out=m10, in0=ss, scalar1=1.0 / HD, scalar2=RMS_EPS,
                op0=ALU.mult, op1=ALU.add,
            )
            # rsqrt on DVE: quake seed + one Newton step (keeps ACT exp-only
            # so a single activation table set serves the whole kernel)
            I32 = mybir.dt.int32
            hsh = work.tile([P, 2, 5], I32, name=f"hs{u}", tag="hsh")
            nc.vector.tensor_scalar(
                out=hsh, in0=m10.bitcast(I32), scalar1=1, scalar2=None,
                op0=ALU.logical_shift_right,
            )
            r0i = work.tile([P, 2, 5], I32, name=f"r0{u}", tag="r0i")
            nc.vector.tensor_scalar(
                out=r0i, in0=hsh, scalar1=-1, scalar2=0x5F3759DF,
                op0=ALU.mult, op1=ALU.add,
            )
            r0 = r0i.bitcast(F32)
            t1 = work.tile([P, 2, 5], F32, name=f"t1{u}", tag="t1")
            nc.vector.tensor_mul(t1, r0, r0)
            nc.vector.tensor_mul(t1, m10, t1)
            nc.vector.tensor_scalar(
                out=t1, in0=t1, scalar1=-0.5, scalar2=1.5, op0=ALU.mult, op1=ALU.add,
            )
            r10 = work.tile([P, 2, 5], BF16, name=f"r10{u}", tag="r10")
            nc.vector.tensor_mul(r10, r0, t1)
            nc.vector.tensor_mul(
                r10[:, :, 0:NH], r10[:, :, 0:NH],
                qg8_sb[:, None, :].broadcast_to([P, 2, NH]),
            )
            qks = work.tile([P, 2, 5, HD], BF16, name=f"qks{u}", tag="qks")
            nc.vector.tensor_mul(
                qks, qc, r10[:, :, :, None].broadcast_to([P, 2, 5, HD])
            )
            tcos = work.tile([P, 2, 5, HD], BF16, name=f"tcos{u}", tag="tcos")
            nc.vector.tensor_mul(
                tcos, qks,
                cos_sb[:, i0 : i0 + 2, None, :].broadcast_to([P, 2, 5, HD]),
            )
            tsin = work.tile([P, 2, 5, HD], BF16, name=f"tsin{u}", tag="tsin")
            qks_swap = qks.rearrange("p u s (h w) -> p u s h w", h=2)[:, :, :, ::-1, :]
            sin_b = (
                sin_sb[:, i0 : i0 + 2, None, :]
                .broadcast_to([P, 2, 5, HD])
                .rearrange("p u s (h w) -> p u s h w", h=2)
            )
            nc.vector.tensor_mul(
                tsin.rearrange("p u s (h w) -> p u s h w", h=2), qks_swap, sin_b
            )
            for k in range(2):
                rot = work.tile([P, 6, HD], BF16, name=f"rot{u}{k}", tag=f"rot{k}")
                nc.vector.tensor_add(rot[:, 0:5], tcos[:, k], tsin[:, k])
                nc.vector.tensor_add(rot[:, 5:6], tcos[:, k, 4:5], tsin[:, k, 4:5])
                teng = nc.scalar if u < 2 else nc.sync
                teng.dma_start_transpose(
                    out=qkT_sb[:, i0 + k, :, :],
                    in_=rot.rearrange("p a b -> p (a b)"),
                )

        # ---------------- phase 2: attention for (pair c, block j) -------
        mask2 = mask_sb[:, None, :].broadcast_to([P, 2, P])

        def attention(c, j, extras=()):
            nt = 4 * (j + 1)
            extras = list(extras)
            y_ps = [
                yp.tile([HD + 1, JW], F32, name=f"y{c}{j}a", tag="yA"),
                yp.tile([HD + 1, JW], F32, name=f"y{c}{j}b", tag="yB"),
            ]

            def scores(t):
                m = t - 4 * j
                qlo = P * m if m >= 0 else 0
                st = stp.tile([P, 2, JW], F32, name=f"s{c}{j}{t}", tag="st")
                for h in range(2):
                    base = HD * h
                    nc.tensor.matmul(
                        st[:, h, qlo:JW],
                        lhsT=qkT_sb[base : base + HD, t, 2, :],
                        rhs=qkT_sb[
                            base : base + HD, 4 * j + (qlo // P) : 4 * (j + 1), c, :
                        ],
                        start=True,
                        stop=True,
                        tile_position=(base, 0),
                    )
                return st, qlo

            # software pipeline: scores(next) is emitted (and thus queued on
            # the PE) before PV(cur), so the PE streams scores while the ACT
            # engine runs exp(cur) instead of stalling in-order behind PV.
            order = list(range(nt))
            cur = scores(order[0])
            for oi, t in enumerate(order):
                st, qlo = cur
                p2 = p2p.tile([P, 2, JW], BF16, name=f"p{c}{j}{t}", tag="p2")
                nc.scalar.activation(p2[:, :, qlo:JW], st[:, :, qlo:JW], ACT.Exp)
                if t - 4 * j >= 0:
                    nc.vector.tensor_mul(
                        p2[:, :, qlo : qlo + P], p2[:, :, qlo : qlo + P], mask2
                    )
                if oi + 1 < nt:
                    cur = scores(order[oi + 1])
                for h in range(2):
                    nc.tensor.matmul(
                        y_ps[h][:, qlo:JW],
                        lhsT=v_sb[:, t, :],
                        rhs=p2[:, h, qlo:JW],
                        start=(oi == 0),
                        stop=(oi == nt - 1),
                    )
                if extras:
                    extras.pop(0)()
            # unnormalized y + denominators out of PSUM; the two copies run
            # on different engines so y_ps frees fast at block boundaries.
            stgA = work.tile([HD + 1, JW], BF16, name=f"ysa{c}{j}", tag="ystgA")
            nc.scalar.copy(stgA, y_ps[0])
            stgB = work.tile([HD + 1, JW], BF16, name=f"ysb{c}{j}", tag="ystgB")
            nc.vector.tensor_copy(stgB, y_ps[1])
            nc.sync.dma_start(out=y_sb[0:HD, c, ts(j, JW)], in_=stgA[0:HD, :])
            nc.sync.dma_start(out=y_sb[HD:P, c, ts(j, JW)], in_=stgB[0:HD, :])
            for h, stg in ((0, stgA), (1, stgB)):
                base = 32 * c + 4 * h
                nc.sync.dma_start(
                    out=den_sb[base : base + 4, j, :],
                    in_=stg[HD : HD + 1, :],
                )

        # ---------------- phase 3: normalize + output projection ---------
        y2s = {}

        def normpre(c, j):
            rden = work.tile([8, P], F32, name=f"rd{c}{j}", tag="rden")
            nc.vector.reciprocal(rden, den_sb[32 * c : 32 * c + 8, j, :])
            rdb = work.tile([8, P], BF16, name=f"rb{c}{j}", tag="rdb")
            nc.vector.tensor_copy(rdb, rden)
            rbc = mmp.tile([P, 512], F32, name=f"rbc{c}{j}", tag="mm")
            for qq in range(NJ):
                nc.tensor.matmul(
                    rbc[:, ts(qq, P)],
                    lhsT=selq_sb[:, qq, :],
                    rhs=rdb,
                    start=True,
                    stop=True,
                )
            y2 = work.tile([P, JW], BF16, name=f"y2{c}{j}", tag=f"y2_{c}")
            nc.vector.tensor_mul(y2, y_sb[:, c, ts(j, JW)], rbc)
            y2s[(c, j)] = y2

        def outproj(j):
            for mc in range(D // P):
                op = mmp.tile([P, 512], F32, name=f"op{mc}{j}", tag="mm")
                for c in range(2):
                    nc.tensor.matmul(
                        op,
                        lhsT=wp_sb[:, c, ts(mc, P)],
                        rhs=y2s[(c, j)],
                        start=(c == 0),
                        stop=(c == 1),
                    )
                ob = work.tile([P, JW], BF16, name=f"ob{mc}{j}", tag="ob")
                if j == 3 and mc % 2 == 0:
                    nc.scalar.copy(ob, op)
                else:
                    nc.vector.tensor_copy(ob, op)
                nc.sync.dma_start(out=ypt[ts(mc, P), ts(j, JW)], in_=ob)

        # ---------------- emission schedule ------------------------------
        for u in range(4):
            phase1(u)
        attention(0, 0); phase1(4)
        attention(1, 0); phase1(5); normpre(0, 0)
        attention(0, 1); phase1(6); normpre(1, 0)
        attention(1, 1); phase1(7); normpre(0, 1); outproj(0)
        attention(0, 2); normpre(1, 1)
        attention(1, 2); normpre(0, 2); outproj(1)
        attention(0, 3); normpre(1, 2)
        attention(1, 3); normpre(0, 3); outproj(2)
        normpre(1, 3); outproj(3)

_PROG = None


def _get_program():
    global _PROG
    if _PROG is None:
        _PROG = _build_program()
    return _PROG


def _host_tables():
    inv_freq = 1.0 / (ROPE_BASE ** (np.arange(0, HD, 2, dtype=np.float32) / HD))
    t = np.arange(S, dtype=np.float32)
    freqs = t[:, None] * inv_freq[None, :].astype(np.float32)  # [S, 32]
    cosf = np.cos(freqs).astype(np.float32)
    sinf = np.sin(freqs).astype(np.float32)
    cosd = np.concatenate([cosf, cosf], axis=1)          # [S, 64]
    sind = np.concatenate([sinf, -sinf], axis=1)         # [S, 64] sign baked
    cosn = np.ascontiguousarray(
        cosd.reshape(NST, P, HD).transpose(1, 0, 2).reshape(P, NST * HD)
    ).astype(bfloat16)
    sinpm = np.ascontiguousarray(
        sind.reshape(NST, P, HD).transpose(1, 0, 2).reshape(P, NST * HD)
    ).astype(bfloat16)
    p_idx = np.arange(P)[:, None]
    c_idx = np.arange(P)[None, :]
    maskt = (c_idx >= p_idx).astype(bfloat16)            # [128, 128]
    # selectors: selq[r, qq, p] = 1 iff r == 4*(p//64) + qq
    selq = np.zeros((8, NJ, P), dtype=bfloat16)
    for qq in range(NJ):
        for p in range(P):
            selq[4 * (p // HD) + qq, qq, p] = 1.0
    selq = np.ascontiguousarray(selq.reshape(8, NJ * P))
    return cosn, sinpm, maskt, selq


def _in_maps(x, Wq, Wk, Wv, Wproj, q_gain):
    cosn, sinpm, maskt, selq = _host_tables()
    maps = []
    for core in range(NC):
        b, g = divmod(core, KV)
        xTf = x[b].T.astype(bfloat16)  # [D, S]
        xTc = np.ascontiguousarray(
            np.concatenate([xTf[:, : S // 2], xTf[:, S // 2 :]], axis=0)
        )  # [2D, S/2] contiguous halves
        wqkv = np.ascontiguousarray(
            np.concatenate(
                [
                    Wq[g * GD : (g + 1) * GD].T,
                    Wk[g * HD : (g + 1) * HD].T,
                    Wv[g * HD : (g + 1) * HD].T,
                ],
                axis=1,
            )
        ).astype(bfloat16)  # [D, 384]
        wsl = Wproj[:, g * GD : (g + 1) * GD].T.reshape(NH, HD, D)  # [head, d, m]
        wp2 = np.ascontiguousarray(
            np.stack(
                [
                    np.concatenate([wsl[0], wsl[1]], axis=0),
                    np.concatenate([wsl[2], wsl[3]], axis=0),
                ],
                axis=1,
            ).reshape(P, 2 * D)
        ).astype(bfloat16)
        qg8 = np.ascontiguousarray(
            (q_gain[g * NH : (g + 1) * NH] / 8.0).reshape(1, NH)
        ).astype(bfloat16)
        maps.append(
            {
                "xT": xTc,
                "wqkv": wqkv,
                "wp2": wp2,
                "cosn": cosn,
                "sinpm": sinpm,
                "maskt": maskt,
                "selq": selq,
                "qg8": qg8,
            }
        )
    return maps


def kernel(x, Wq, Wk, Wv, Wproj, q_gain, _collect=None):
    x = np.asarray(x, dtype=np.float32)
    Wq = np.asarray(Wq, dtype=np.float32)
    Wk = np.asarray(Wk, dtype=np.float32)
    Wv = np.asarray(Wv, dtype=np.float32)
    Wproj = np.asarray(Wproj, dtype=np.float32)
    q_gain = np.asarray(q_gain, dtype=np.float32)

    nc = _get_program()
    maps = _in_maps(x, Wq, Wk, Wv, Wproj, q_gain)
    res = run_bass_kernel_spmd(nc, maps, core_ids=list(range(NC)))
    if _collect is not None:
        _collect.append(res)

    out = np.zeros((B, S, D), dtype=np.float64)
    for core in range(NC):
        b, _ = divmod(core, KV)
        out[b] += res.results[core]["ypt"].T.astype(np.float64)
    return out.astype(np.float32)


# revision 30
# speedup vs baseline: 1.1389x; 1.1389x over previous
"""Trainium2 Bass kernel for a causal self-attention block (GQA + per-head
RMS-norm + RoPE + learned q-gain), sharded over 8 NeuronCores.

Sharding: data-parallel over batch (B=2) x tensor-parallel over KV groups
(4 groups of 4 query heads). core = b*4 + g. Each core computes full
attention for its 4 heads and a partial output projection (its 256 in-dims
of Wproj); the host sums the 4 partials per batch element.

v3 design:
- bf16 operands everywhere (fp32 PSUM accumulate).
- Scores as S^T[k, q] = K @ Q^T with heads PAIRED: two K=64 matmuls run
  concurrently in the PE via tile_position row groups.
- Causal trimming at 128-column granularity; one [128,128] step mask for
  the diagonal boundary.
- Phase-1 transposes: ONE fused DMA XBAR transpose per s-tile
  ([128,384] -> [128,3,128] block transpose) writing q-pair/k slabs
  directly into the merged qkT layout. Zero PE/DVE cost, 16 DMA ops.
- RMS rsqrt via Ln+Exp so the whole kernel uses a single ACT table set
  (natural_log_exp_and_others covers Exp/Ln/Square/Copy).
- Softmax denominators ride the PV matmul as a ones-column, gathered
  16-per-partition, one wide reciprocal, broadcast back via K=16 selector
  matmuls, normalization fused into the output-projection preamble.
- Phase interleaving keeps every engine queue dense.
"""

import math

import numpy as np
import ml_dtypes

import concourse.bacc as bacc
import concourse.tile as tile
from concourse import mybir
from concourse.bass import ts
from concourse.bass_utils import run_bass_kernel_spmd

# Problem dims (hardcoded per contract).
B, S, D, H, KV, HD = 2, 2048, 1024, 16, 4, 64
NH = H // KV          # 4 query heads per core (one KV group)
GD = NH * HD          # 256 out-dims of Wq per group
NQKV = GD + 2 * HD    # 384
P = 128               # partitions
NST = S // P          # 16 sequence tiles
JW = 512              # query-block width for attention
NJ = S // JW          # 4 query blocks
NC = 8                # cores
ROPE_BASE = 10000.0
RMS_EPS = 1.1920929e-07
F32 = mybir.dt.float32
BF16 = mybir.dt.bfloat16
AXX = mybir.AxisListType.X
ACT = mybir.ActivationFunctionType
ALU = mybir.AluOpType

bfloat16 = ml_dtypes.bfloat16


def _build_program():
    nc = bacc.Bacc("TRN2", target_bir_lowering=False, debug=False)

    xT = nc.dram_tensor("xT", [2 * D, S // 2], BF16, kind="ExternalInput").ap()
    wqkv = nc.dram_tensor("wqkv", [D, NQKV], BF16, kind="ExternalInput").ap()
    wp2 = nc.dram_tensor("wp2", [P, 2 * D], BF16, kind="ExternalInput").ap()
    cosn = nc.dram_tensor("cosn", [P, NST * HD], BF16, kind="ExternalInput").ap()
    sinpm = nc.dram_tensor("sinpm", [P, NST * HD], BF16, kind="ExternalInput").ap()
    maskt = nc.dram_tensor("maskt", [P, P], BF16, kind="ExternalInput").ap()
    selq = nc.dram_tensor("selq", [8, NJ * P], BF16, kind="ExternalInput").ap()
    qg8 = nc.dram_tensor("qg8", [1, NH], BF16, kind="ExternalInput").ap()
    ypt = nc.dram_tensor("ypt", [D, S], BF16, kind="ExternalOutput").ap()

    with tile.TileContext(nc) as tc:
        _body(tc, xT, wqkv, wp2, cosn, sinpm, maskt, selq, qg8, ypt)
    nc.compile()
    return nc


def _body(tc, xT, wqkv, wp2, cosn, sinpm, maskt, selq, qg8, ypt):
    nc = tc.nc

    with (
        tc.tile_pool(name="consts", bufs=1) as consts,
        tc.tile_pool(name="work", bufs=4) as work,
        tc.tile_pool(name="p2p", bufs=6) as p2p,
        tc.tile_pool(name="mmp", bufs=2, space="PSUM") as mmp,
        tc.tile_pool(name="stp", bufs=2, space="PSUM") as stp,
        tc.tile_pool(name="yp", bufs=1, space="PSUM") as yp,
    ):
        # ---------------- persistent SBUF state ----------------
        xT_sb = [
            [consts.tile([P, S // 2], BF16, name=f"xT_sb{c}_{hf}") for hf in range(2)]
            for c in range(8)
        ]
        wqkv_sb = consts.tile([P, 8, NQKV], BF16, name="wqkv_sb")
        wp_sb = consts.tile([P, 2, D], BF16, name="wp_sb")
        cos_sb = consts.tile([P, NST, HD], BF16, name="cos_sb")
        sin_sb = consts.tile([P, NST, HD], BF16, name="sin_sb")
        mask_sb = consts.tile([P, P], BF16, name="mask_sb")
        selq_sb = consts.tile([8, NJ, P], BF16, name="selq_sb")
        qg8_sb = consts.tile([P, NH], BF16, name="qg8_sb")
        # merged attention operand layout: [:, i, 0/1, :] = qT pair c tile i,
        # [:, i, 2, :] = kT duplicated into both partition halves.
        qkT_sb = consts.tile([P, NST, 3, P], BF16, name="qkT_sb")
        v_sb = consts.tile([P, NST, HD + 1], BF16, name="v_sb")
        y_sb = consts.tile([P, 2, S], BF16, name="y_sb")
        den_sb = consts.tile([64, NJ, P], BF16, name="den_sb")

        nc.sync.dma_start(out=wqkv_sb, in_=wqkv.rearrange("(c p) n -> p c n", p=P))
        for hf in range(2):
            for c in range(8):
                eng = nc.sync if c % 2 == 0 else nc.scalar
                base = hf * D + c * P
                eng.dma_start(out=xT_sb[c][hf], in_=xT[base : base + P, :])
        nc.scalar.dma_start(out=cos_sb, in_=cosn.rearrange("p (t f) -> p t f", f=HD))
        nc.scalar.dma_start(out=sin_sb, in_=sinpm.rearrange("p (t f) -> p t f", f=HD))
        nc.sync.dma_start(out=wp_sb, in_=wp2.rearrange("p (c m) -> p c m", c=2))
        nc.sync.dma_start(out=mask_sb, in_=maskt)
        nc.sync.dma_start(
            out=selq_sb, in_=selq.rearrange("r (j p) -> r j p", j=NJ)
        )
        nc.gpsimd.dma_start(out=qg8_sb, in_=qg8.to_broadcast([P, NH]))
        nc.vector.memset(v_sb[:, :, HD : HD + 1], 1.0)

        # ------------ phase 1: QKV + RMS + RoPE + transpose (tile pairs) --
        def phase1(u):
            i0 = 2 * u
            qkvs = []
            for k in range(2):
                qkv = mmp.tile([P, 512], F32, name=f"qkv{u}{k}", tag="mm")
                for cc in range(8):
                    nc.tensor.matmul(
                        qkv[:, 0:NQKV],
                        lhsT=xT_sb[cc][(i0 + k) // 8][:, ts((i0 + k) % 8, P)],
                        rhs=wqkv_sb[:, cc, :],
                        start=(cc == 0),
                        stop=(cc == 7),
                    )
                qkvs.append(qkv)
            # startup pairs borrow the idle ACT engine for PSUM extraction
            # (Square/Copy share the exp table set: no table reload)
            cpy = nc.scalar.copy if u < 4 else nc.vector.tensor_copy
            qc = work.tile([P, 2, 5, HD], BF16, name=f"qc{u}", tag="qc")
            for k in range(2):
                cpy(v_sb[:, i0 + k, 0:HD], qkvs[k][:, 5 * HD : NQKV])
                cpy(
                    qc[:, k], qkvs[k][:, 0 : 5 * HD].rearrange("p (s d) -> p s d", d=HD)
                )
            sq = work.tile([P, 2, 5, HD], BF16, name=f"sq{u}", tag="sq")
            if u < 4:
                nc.scalar.square(sq, qc)
            else:
                nc.vector.tensor_mul(sq, qc, qc)
            ss = work.tile([P, 2, 5], F32, name=f"ss{u}", tag="ss")
            nc.vector.reduce_sum(ss, sq, axis=AXX)
            m10 = work.tile([P, 2, 5], F32, name=f"m10{u}", tag="m10")
            nc.vector.tensor_scalar(
                out=m10, in0=ss, scalar1=1.0 / HD, scalar2=RMS_EPS,
                op0=ALU.mult, op1=ALU.add,
            )
            # rsqrt on DVE: quake seed + one Newton step (keeps ACT exp-only
            # so a single activation table set serves the whole kernel)
            I32 = mybir.dt.int32
            hsh = work.tile([P, 2, 5], I32, name=f"hs{u}", tag="hsh")
            nc.vector.tensor_scalar(
                out=hsh, in0=m10.bitcast(I32), scalar1=1, scalar2=None,
                op0=ALU.logical_shift_right,
            )
            r0i = work.tile([P, 2, 5], I32, name=f"r0{u}", tag="r0i")
            nc.vector.tensor_scalar(
                out=r0i, in0=hsh, scalar1=-1, scalar2=0x5F3759DF,
                op0=ALU.mult, op1=ALU.add,
            )
            r0 = r0i.bitcast(F32)
            t1 = work.tile([P, 2, 5], F32, name=f"t1{u}", tag="t1")
            nc.vector.tensor_mul(t1, r0, r0)
            nc.vector.tensor_mul(t1, m10, t1)
            nc.vector.tensor_scalar(
                out=t1, in0=t1, scalar1=-0.5, scalar2=1.5, op0=ALU.mult, op1=ALU.add,
            )
            r10 = work.tile([P, 2, 5], BF16, name=f"r10{u}", tag="r10")
            nc.vector.tensor_mul(r10, r0, t1)
            nc.vector.tensor_mul(
                r10[:, :, 0:NH], r10[:, :, 0:NH],
                qg8_sb[:, None, :].broadcast_to([P, 2, NH]),
            )
            qks = work.tile([P, 2, 5, HD], BF16, name=f"qks{u}", tag="qks")
            nc.vector.tensor_mul(
                qks, qc, r10[:, :, :, None].broadcast_to([P, 2, 5, HD])
            )
            tcos = work.tile([P, 2, 5, HD], BF16, name=f"tcos{u}", tag="tcos")
            nc.vector.tensor_mul(
                tcos, qks,
                cos_sb[:, i0 : i0 + 2, None, :].broadcast_to([P, 2, 5, HD]),
            )
            tsin = work.tile([P, 2, 5, HD], BF16, name=f"tsin{u}", tag="tsin")
            qks_swap = qks.rearrange("p u s (h w) -> p u s h w", h=2)[:, :, :, ::-1, :]
            sin_b = (
                sin_sb[:, i0 : i0 + 2, None, :]
                .broadcast_to([P, 2, 5, HD])
                .rearrange("p u s (h w) -> p u s h w", h=2)
            )
            nc.vector.tensor_mul(
                tsin.rearrange("p u s (h w) -> p u s h w", h=2), qks_swap, sin_b
            )
            for k in range(2):
                rot = work.tile([P, 6, HD], BF16, name=f"rot{u}{k}", tag=f"rot{k}")
                nc.vector.tensor_add(rot[:, 0:5], tcos[:, k], tsin[:, k])
                nc.vector.tensor_add(rot[:, 5:6], tcos[:, k, 4:5], tsin[:, k, 4:5])
                teng = nc.scalar if u < 2 else nc.sync
                teng.dma_start_transpose(
                    out=qkT_sb[:, i0 + k, :, :],
                    in_=rot.rearrange("p a b -> p (a b)"),
                )

        # ---------------- phase 2: attention for (pair c, block j) -------
        mask2 = mask_sb[:, None, :].broadcast_to([P, 2, P])

        def attention(c, j, extras=()):
            nt = 4 * (j + 1)
            extras = list(extras)
            y_ps = [
                yp.tile([HD + 1, JW], F32, name=f"y{c}{j}a", tag="yA"),
                yp.tile([HD + 1, JW], F32, name=f"y{c}{j}b", tag="yB"),
            ]

            def scores(t):
                m = t - 4 * j
                qlo = P * m if m >= 0 else 0
                st = stp.tile([P, 2, JW], F32, name=f"s{c}{j}{t}", tag="st")
                for h in range(2):
                    base = HD * h
                    nc.tensor.matmul(
                        st[:, h, qlo:JW],
                        lhsT=qkT_sb[base : base + HD, t, 2, :],
                        rhs=qkT_sb[
                            base : base + HD, 4 * j + (qlo // P) : 4 * (j + 1), c, :
                        ],
                        start=True,
                        stop=True,
                        tile_position=(base, 0),
                    )
                return st, qlo

            # software pipeline: scores(next) is emitted (and thus queued on
            # the PE) before PV(cur), so the PE streams scores while the ACT
            # engine runs exp(cur) instead of stalling in-order behind PV.
            order = list(range(nt))
            cur = scores(order[0])
            for oi, t in enumerate(order):
                st, qlo = cur
                p2 = p2p.tile([P, 2, JW], BF16, name=f"p{c}{j}{t}", tag="p2")
                nc.scalar.activation(p2[:, :, qlo:JW], st[:, :, qlo:JW], ACT.Exp)
                if t - 4 * j >= 0:
                    nc.vector.tensor_mul(
                        p2[:, :, qlo : qlo + P], p2[:, :, qlo : qlo + P], mask2
                    )
                if oi + 1 < nt:
                    cur = scores(order[oi + 1])
                for h in range(2):
                    nc.tensor.matmul(
                        y_ps[h][:, qlo:JW],
                        lhsT=v_sb[:, t, :],
                        rhs=p2[:, h, qlo:JW],
                        start=(oi == 0),
                        stop=(oi == nt - 1),
                    )
                if extras:
                    extras.pop(0)()
            # unnormalized y + denominators out of PSUM; the two copies run
            # on different engines so y_ps frees fast at block boundaries.
            stgA = work.tile([HD + 1, JW], BF16, name=f"ysa{c}{j}", tag="ystgA")
            nc.scalar.copy(stgA, y_ps[0])
            stgB = work.tile([HD + 1, JW], BF16, name=f"ysb{c}{j}", tag="ystgB")
            nc.vector.tensor_copy(stgB, y_ps[1])
            nc.sync.dma_start(out=y_sb[0:HD, c, ts(j, JW)], in_=stgA[0:HD, :])
            nc.sync.dma_start(out=y_sb[HD:P, c, ts(j, JW)], in_=stgB[0:HD, :])
            for h, stg in ((0, stgA), (1, stgB)):
                base = 32 * c + 4 * h
                nc.sync.dma_start(
                    out=den_sb[base : base + 4, j, :],
                    in_=stg[HD : HD + 1, :],
                )

        # ---------------- phase 3: normalize + output projection ---------
        y2s = {}

        def normpre(c, j):
            rden = work.tile([8, P], F32, name=f"rd{c}{j}", tag="rden")
            nc.vector.reciprocal(rden, den_sb[32 * c : 32 * c + 8, j, :])
            rdb = work.tile([8, P], BF16, name=f"rb{c}{j}", tag="rdb")
            nc.vector.tensor_copy(rdb, rden)
            rbc = mmp.tile([P, 512], F32, name=f"rbc{c}{j}", tag="mm")
            for qq in range(NJ):
                nc.tensor.matmul(
                    rbc[:, ts(qq, P)],
                    lhsT=selq_sb[:, qq, :],
                    rhs=rdb,
                    start=True,
                    stop=True,
                )
            y2 = work.tile([P, JW], BF16, name=f"y2{c}{j}", tag=f"y2_{c}")
            nc.vector.tensor_mul(y2, y_sb[:, c, ts(j, JW)], rbc)
            y2s[(c, j)] = y2

        def outproj(j):
            for mc in range(D // P):
                op = mmp.tile([P, 512], F32, name=f"op{mc}{j}", tag="mm")
                for c in range(2):
                    nc.tensor.matmul(
                        op,
                        lhsT=wp_sb[:, c, ts(mc, P)],
                        rhs=y2s[(c, j)],
                        start=(c == 0),
                        stop=(c == 1),
                    )
                ob = work.tile([P, JW], BF16, name=f"ob{mc}{j}", tag="ob")
                if j == 3 and mc % 2 == 0:
                    nc.scalar.copy(ob, op)
                else:
                    nc.vector.tensor_copy(ob, op)
                nc.sync.dma_start(out=ypt[ts(mc, P), ts(j, JW)], in_=ob)

        # ---------------- emission schedule ------------------------------
        for u in range(4):
            phase1(u)
        attention(0, 0); phase1(4)
        attention(1, 0); phase1(5); normpre(0, 0)
        attention(0, 1); phase1(6); normpre(1, 0)
        attention(1, 1); phase1(7); normpre(0, 1); outproj(0)
        attention(0, 2); normpre(1, 1)
        attention(1, 2); normpre(0, 2); outproj(1)
        attention(0, 3); normpre(1, 2)
        attention(1, 3); normpre(0, 3); outproj(2)
        normpre(1, 3); outproj(3)

_PROG = None


def _get_program():
    global _PROG
    if _PROG is None:
        _PROG = _build_program()
    return _PROG


def _host_tables():
    inv_freq = 1.0 / (ROPE_BASE ** (np.arange(0, HD, 2, dtype=np.float32) / HD))
    t = np.arange(S, dtype=np.float32)
    freqs = t[:, None] * inv_freq[None, :].astype(np.float32)  # [S, 32]
    cosf = np.cos(freqs).astype(np.float32)
    sinf = np.sin(freqs).astype(np.float32)
    cosd = np.concatenate([cosf, cosf], axis=1)          # [S, 64]
    sind = np.concatenate([sinf, -sinf], axis=1)         # [S, 64] sign baked
    cosn = np.ascontiguousarray(
        cosd.reshape(NST, P, HD).transpose(1, 0, 2).reshape(P, NST * HD)
    ).astype(bfloat16)
    sinpm = np.ascontiguousarray(
        sind.reshape(NST, P, HD).transpose(1, 0, 2).reshape(P, NST * HD)
    ).astype(bfloat16)
    p_idx = np.arange(P)[:, None]
    c_idx = np.arange(P)[None, :]
    maskt = (c_idx >= p_idx).astype(bfloat16)            # [128, 128]
    # selectors: selq[r, qq, p] = 1 iff r == 4*(p//64) + qq
    selq = np.zeros((8, NJ, P), dtype=bfloat16)
    for qq in range(NJ):
        for p in range(P):
            selq[4 * (p // HD) + qq, qq, p] = 1.0
    selq = np.ascontiguousarray(selq.reshape(8, NJ * P))
    return cosn, sinpm, maskt, selq


def _in_maps(x, Wq, Wk, Wv, Wproj, q_gain):
    cosn, sinpm, maskt, selq = _host_tables()
    maps = []
    for core in range(NC):
        b, g = divmod(core, KV)
        xTf = x[b].T.astype(bfloat16)  # [D, S]
        xTc = np.ascontiguousarray(
            np.concatenate([xTf[:, : S // 2], xTf[:, S // 2 :]], axis=0)
        )  # [2D, S/2] contiguous halves
        wqkv = np.ascontiguousarray(
            np.concatenate(
                [
                    Wq[g * GD : (g + 1) * GD].T,
                    Wk[g * HD : (g + 1) * HD].T,
                    Wv[g * HD : (g + 1) * HD].T,
                ],
                axis=1,
            )
        ).astype(bfloat16)  # [D, 384]
        wsl = Wproj[:, g * GD : (g + 1) * GD].T.reshape(NH, HD, D)  # [head, d, m]
        wp2 = np.ascontiguousarray(
            np.stack(
                [
                    np.concatenate([wsl[0], wsl[1]], axis=0),
                    np.concatenate([wsl[2], wsl[3]], axis=0),
                ],
                axis=1,
            ).reshape(P, 2 * D)
        ).astype(bfloat16)
        qg8 = np.ascontiguousarray(
            (q_gain[g * NH : (g + 1) * NH] / 8.0).reshape(1, NH)
        ).astype(bfloat16)
        maps.append(
            {
                "xT": xTc,
                "wqkv": wqkv,
                "wp2": wp2,
                "cosn": cosn,
                "sinpm": sinpm,
                "maskt": maskt,
                "selq": selq,
                "qg8": qg8,
            }
        )
    return maps


def kernel(x, Wq, Wk, Wv, Wproj, q_gain, _collect=None):
    x = np.asarray(x, dtype=np.float32)
    Wq = np.asarray(Wq, dtype=np.float32)
    Wk = np.asarray(Wk, dtype=np.float32)
    Wv = np.asarray(Wv, dtype=np.float32)
    Wproj = np.asarray(Wproj, dtype=np.float32)
    q_gain = np.asarray(q_gain, dtype=np.float32)

    nc = _get_program()
    maps = _in_maps(x, Wq, Wk, Wv, Wproj, q_gain)
    res = run_bass_kernel_spmd(nc, maps, core_ids=list(range(NC)))
    if _collect is not None:
        _collect.append(res)

    out = np.zeros((B, S, D), dtype=np.float64)
    for core in range(NC):
        b, _ = divmod(core, KV)
        out[b] += res.results[core]["ypt"].T.astype(np.float64)
    return out.astype(np.float32)
